# revision 3
# baseline (speedup 1.0000x reference)
"""AdaptConv2d Trainium2 kernel v2: per-sample adapted 1x1 conv (SE-modulated).

Reference computation (B=32, C=O=256, H=W=56, HID=16):
    pooled = mean(x, (2,3))                      [B, C]
    hid    = silu(pooled @ w_se1.T)              [B, 16]
    adapt  = (hid @ w_se_out.T).reshape(B,O,C)   [B, O, C]
    out[b] = (w_conv + adapt[b]) @ x[b]          [B, O, H*W]

Distribution: data-parallel over batch, 4 samples per core on 8 cores.

v2 pipeline (vs v1 two-phase):
  - x loaded in per-sample quarters alternating both HWDGE rings,
    samples 0,1 FIRST, then w_se in column chunks, then samples 2,3.
    Pools (free-dim reduce) per quarter on ACT/DVE right behind each
    arrival, so pooled[b] is ready ~1.5us after x[b] lands.
  - SE/adapt runs in TWO 2-sample groups: group A {0,1} starts its
    adapt matmul while samples 2,3 are still loading; its 16 psum
    tiles stream against the arriving w_se column chunks.
  - adapt rows are (bl*8+g) so each sample's 8 rows are contiguous;
    reshape to per-sample lhsT [c, k*256+o] goes through a DRAM bounce
    (1 store + 2 strided loads per sample) instead of 32 tiny
    SBUF->SBUF DMAs; w_conv (host-prepped in lhsT layout) is added
    in-place on DVE.
  - main GEMM per sample [O,C]@[C,HW] in bf16, K-contiguous psum
    groups of 3, evictions alternate DVE/ACT, 784KB output stores
    alternate rings. Group B's adapt MMs are slotted into the PE
    stream between sample 1 and sample 2 GEMMs.
  - junk matmuls on early arriving quarters keep the PE HAM clock
    warm before the first real matmul work.
"""

import numpy as np

B_PER_CORE = 4
N_CORES = 8
C = 256
O = 256
HW = 3136
HWQ = HW // 2  # 1568, quarter of a sample (half of one k-half)
HID = 16
P = 128
G = 8
NT = 448  # free-dim tile of the main GEMM (7 * 448 = 3136)
NN = HW // NT
ADN = C * O // G  # 8192 streaming columns for adapt
ADT = ADN // 512  # 16 psum tiles
WSE_CHUNK = 2048  # w_se column chunk per load DMA
JUNK = {0: 2, 1: 3, 2: 2, 3: 0}  # warmup matmuls per arriving quarter, by sample


def build_nc():
    from concourse import bacc, tile, mybir

    f32 = mybir.dt.float32
    bf16 = mybir.dt.bfloat16

    nc = bacc.Bacc("TRN2", target_bir_lowering=False, debug=False)

    x_d = nc.dram_tensor("x", [B_PER_CORE, P, 2 * HW], bf16, kind="ExternalInput")
    wse_d = nc.dram_tensor("w_se", [P, ADN], bf16, kind="ExternalInput")
    wconv_d = nc.dram_tensor("w_conv", [P, 2 * O], bf16, kind="ExternalInput")
    wse1_d = nc.dram_tensor("w_se1", [P, 2 * P], f32, kind="ExternalInput")
    mask_d = nc.dram_tensor("bd_mask", [P, 2 * G], bf16, kind="ExternalInput")
    adsc_d = nc.dram_tensor("adsc", [2, 2 * G, ADN], bf16, kind="Internal")
    out_d = nc.dram_tensor("out", [B_PER_CORE, 2, P, HW], bf16, kind="ExternalOutput")

    with tile.TileContext(nc) as tc:
        with (
            tc.tile_pool(name="xp", bufs=B_PER_CORE) as xp,
            tc.tile_pool(name="wsep", bufs=1) as wsep,
            tc.tile_pool(name="consts", bufs=1) as consts,
            tc.tile_pool(name="adp", bufs=1) as adp,
            tc.tile_pool(name="wbp", bufs=1) as wbp,
            tc.tile_pool(name="small", bufs=1) as small,
            tc.tile_pool(name="scratch", bufs=2) as scratchp,
            tc.tile_pool(name="stage", bufs=3) as stagep,
            tc.tile_pool(name="ps_ad", bufs=2, space="PSUM") as ps_ad,
            tc.tile_pool(name="ps_z", bufs=1, space="PSUM") as ps_z,
            tc.tile_pool(name="ps_mm", bufs=4, space="PSUM") as ps_mm,
        ):
            junk_ps = ps_mm.tile([P, 512], f32, tag="junk", bufs=1)

            # ---- consts (small, ahead of x on both rings) ----
            wconv_sb = consts.tile([P, 2 * O], bf16, tag="wconv")
            nc.sync.dma_start(out=wconv_sb[:], in_=wconv_d.ap()[:])
            wse1_sb = consts.tile([P, 2 * P], f32, tag="wse1")
            nc.scalar.dma_start(out=wse1_sb[:], in_=wse1_d.ap()[:])
            mask_sb = consts.tile([P, 2 * G], bf16, tag="mask")
            nc.scalar.dma_start(out=mask_sb[:], in_=mask_d.ap()[:])

            # prepay the sigmoid LUT load while DMAs stream
            lutw = small.tile([P, 1], f32, tag="lutw")
            nc.scalar.activation(
                lutw[:], wse1_sb[:, 0:1], mybir.ActivationFunctionType.Sigmoid
            )

            # pooled_q[k][:, b*2+j] = partial sum of x[b] k-half, hw-half j
            pooled_q = [
                small.tile(
                    [P, 2 * B_PER_CORE], f32, tag=f"poolq{k}", name=f"poolq{k}"
                )
                for k in range(2)
            ]

            x_tiles = [
                xp.tile([P, 2 * HW], bf16, tag="x", name=f"xt{b}")
                for b in range(B_PER_CORE)
            ]
            wse_sb = wsep.tile([P, ADN], bf16)

            def load_quarters(b):
                for q in range(4):
                    k, j = q // 2, q % 2
                    off = k * HW + j * HWQ
                    eng = nc.sync if q % 2 == 0 else nc.scalar
                    eng.dma_start(
                        out=x_tiles[b][:, off : off + HWQ],
                        in_=x_d.ap()[b][:, off : off + HWQ],
                    )
                    for _ in range(JUNK[b]):
                        nc.tensor.matmul(
                            junk_ps[:],
                            lhsT=x_tiles[b][:, off : off + P],
                            rhs=x_tiles[b][:, off : off + 512],
                            start=True,
                            stop=True,
                        )
                    col = b * 2 + j
                    if (q + b) % 2 == 0:
                        sca = scratchp.tile(
                            [P, HWQ], bf16, tag="poolscratch", name=f"psc{b}{q}"
                        )
                        nc.scalar.activation(
                            out=sca[:],
                            in_=x_tiles[b][:, off : off + HWQ],
                            func=mybir.ActivationFunctionType.Copy,
                            accum_out=pooled_q[k][:, col : col + 1],
                        )
                    else:
                        nc.vector.tensor_reduce(
                            out=pooled_q[k][:, col : col + 1],
                            in_=x_tiles[b][:, off : off + HWQ],
                            axis=mybir.AxisListType.X,
                            op=mybir.AluOpType.add,
                        )

            # ---- loads: x0, x1, then w_se chunks, then x2, x3 ----
            load_quarters(0)
            load_quarters(1)
            for ci in range(ADN // WSE_CHUNK):
                eng = nc.sync if ci % 2 == 0 else nc.scalar
                eng.dma_start(
                    out=wse_sb[:, ci * WSE_CHUNK : (ci + 1) * WSE_CHUNK],
                    in_=wse_d.ap()[:, ci * WSE_CHUNK : (ci + 1) * WSE_CHUNK],
                )
            load_quarters(2)
            load_quarters(3)

            ev = 0  # eviction engine round-robin

            def se_adapt_group(grp):
                """SE chain + adapt matmul for samples (2*grp, 2*grp+1).

                Produces adapt_g [16, 8192]: row bl*8+g holds
                adapt[b=2*grp+bl][c = g*32+cl, o] at col cl*256+o.
                """
                nonlocal ev
                # pooled_sum per k (need both k halves separately)
                psum_k = [
                    small.tile([P, 2], f32, tag=f"psum{grp}k{k}", name=f"ps{grp}{k}")
                    for k in range(2)
                ]
                lo = grp * 4
                for k in range(2):
                    nc.vector.tensor_tensor(
                        out=psum_k[k][:],
                        in0=pooled_q[k][:, lo : lo + 3 : 2],
                        in1=pooled_q[k][:, lo + 1 : lo + 4 : 2],
                        op=mybir.AluOpType.add,
                    )
                z_ps = ps_z.tile([P, 2], f32, tag="z", name=f"z{grp}", bufs=1)
                for k in range(2):
                    nc.tensor.matmul(
                        z_ps[:],
                        lhsT=wse1_sb[:, k * P : (k + 1) * P],
                        rhs=psum_k[k][:],
                        start=(k == 0),
                        stop=(k == 1),
                    )
                sig = small.tile([P, 2], f32, tag=f"sig{grp}", name=f"sig{grp}")
                nc.scalar.activation(
                    sig[:], z_ps[:], mybir.ActivationFunctionType.Sigmoid
                )
                zs = small.tile([P, 2], f32, tag=f"zs{grp}", name=f"zs{grp}")
                nc.vector.tensor_tensor(
                    out=zs[:], in0=sig[:], in1=z_ps[:], op=mybir.AluOpType.mult
                )
                # bd[(g,h), bl*8+g'] = silu(z[h,bl]) * (g==g')
                bd = small.tile([P, 2 * G], bf16, tag=f"bd{grp}", name=f"bd{grp}")
                nc.vector.tensor_tensor(
                    out=bd[:].rearrange("p (bl g) -> p bl g", bl=2, g=G),
                    in0=zs[:].unsqueeze(2).broadcast_to([P, 2, G]),
                    in1=mask_sb[:].rearrange("p (bl g) -> p bl g", bl=2, g=G),
                    op=mybir.AluOpType.mult,
                )
                # adapt matmul: 16 psum tiles of 512 cols
                adapt_g = adp.tile(
                    [2 * G, ADN], bf16, tag=f"adapt{grp}", name=f"adapt{grp}"
                )
                for t in range(ADT):
                    ap_ps = ps_ad.tile(
                        [2 * G, 512], f32, tag="adps", name=f"adps{grp}{t}"
                    )
                    nc.tensor.matmul(
                        ap_ps[:],
                        lhsT=bd[:],
                        rhs=wse_sb[:, t * 512 : (t + 1) * 512],
                        start=True,
                        stop=True,
                    )
                    if ev % 2 == 0:
                        nc.vector.tensor_copy(
                            out=adapt_g[:, t * 512 : (t + 1) * 512], in_=ap_ps[:]
                        )
                    else:
                        nc.scalar.copy(
                            out=adapt_g[:, t * 512 : (t + 1) * 512], in_=ap_ps[:]
                        )
                    ev += 1
                # bounce through DRAM to reshape into per-sample lhsT
                nc.sync.dma_start(out=adsc_d.ap()[grp], in_=adapt_g[:])
                wbs = []
                for bl in range(2):
                    b = grp * 2 + bl
                    wb = wbp.tile([P, 2 * O], bf16, tag=f"wb{b}", name=f"wb{b}")
                    wbs.append(wb)
                    for k in range(2):
                        r0 = bl * 8 + k * 4
                        src = adsc_d.ap()[grp][r0 : r0 + 4, :].rearrange(
                            "gl (cl o) -> gl cl o", cl=32, o=O
                        )
                        nc.scalar.dma_start(
                            out=wb[:, k * O : (k + 1) * O], in_=src
                        )
                    nc.vector.tensor_tensor(
                        out=wb[:],
                        in0=wb[:],
                        in1=wconv_sb[:],
                        op=mybir.AluOpType.add,
                    )
                return wbs

            wbs = [None] * B_PER_CORE
            wbs[0], wbs[1] = se_adapt_group(0)

            def gemm(b, oc):
                nonlocal ev
                stage = stagep.tile([P, HW], bf16, tag="stage", name=f"st{b}{oc}")
                for grp_n in ((0, 1, 2), (3, 4, 5), (6,)):
                    pss = [
                        ps_mm.tile([P, NT], f32, tag="mmps", name=f"ps{b}{oc}{n}")
                        for n in grp_n
                    ]
                    for k in range(2):
                        w_slice = wbs[b][
                            :, k * O + oc * P : k * O + oc * P + P
                        ]
                        for i, n in enumerate(grp_n):
                            nc.tensor.matmul(
                                pss[i][:],
                                lhsT=w_slice,
                                rhs=x_tiles[b][
                                    :, k * HW + n * NT : k * HW + (n + 1) * NT
                                ],
                                start=(k == 0),
                                stop=(k == 1),
                            )
                    for i, n in enumerate(grp_n):
                        if ev % 2 == 0:
                            nc.vector.tensor_copy(
                                out=stage[:, n * NT : (n + 1) * NT], in_=pss[i][:]
                            )
                        else:
                            nc.scalar.copy(
                                out=stage[:, n * NT : (n + 1) * NT], in_=pss[i][:]
                            )
                        ev += 1
                eng = nc.sync if (b * 2 + oc) % 2 == 0 else nc.scalar
                eng.dma_start(out=out_d.ap()[b, oc], in_=stage[:])

            gemm(0, 0)
            gemm(0, 1)
            gemm(1, 0)
            gemm(1, 1)
            wbs[2], wbs[3] = se_adapt_group(1)
            gemm(2, 0)
            gemm(2, 1)
            gemm(3, 0)
            gemm(3, 1)

    nc.compile()
    return nc


def prep_core_inputs(x_shard, w_conv, w_se1, w_se_out):
    """Host-side layout prep for one core. x_shard: [4, 256, 56, 56] f32."""
    import ml_dtypes

    bf16 = ml_dtypes.bfloat16
    b = x_shard.shape[0]
    # x: [b, 128, 2*3136], c = k*128 + p, free = k*3136 + hw
    xr = x_shard.reshape(b, 2, P, HW).transpose(0, 2, 1, 3).reshape(b, P, 2 * HW)
    x_dev = np.ascontiguousarray(xr).astype(bf16)
    # w_se: [(g,h), n] with flat = c*256 + o = g*8192 + n, n = cl*256 + o
    w_r = w_se_out.reshape(O, C, HID).transpose(1, 0, 2)  # [c, o, h]
    w_r = w_r.reshape(G, ADN, HID).transpose(0, 2, 1).reshape(P, ADN)
    wse_dev = np.ascontiguousarray(w_r).astype(bf16)
    # w_conv in per-sample lhsT layout: [c_p, k*256 + o] = w_conv[o, k*128 + c_p]
    wc = w_conv[:, :, 0, 0]  # [O, C]
    wconv_dev = np.ascontiguousarray(
        wc.T.reshape(2, P, O).transpose(1, 0, 2).reshape(P, 2 * O)
    ).astype(bf16)
    # w_se1 replicated for the G h-groups:
    # [p, k*128 + (g*16+h)] = w_se1[h, k*128+p] / 3136
    w1 = (w_se1.T / float(HW)).reshape(2, P, HID)  # [k, p, h]
    w1 = np.broadcast_to(w1[:, :, None, :], (2, P, G, HID)).reshape(2, P, P)
    w1 = np.ascontiguousarray(w1.transpose(1, 0, 2).reshape(P, 2 * P)).astype(
        np.float32
    )
    # bd mask: [(g,h), bl*8 + g'] = 1 if g == g'
    m = np.zeros((G, HID, 2, G), np.float32)
    for g in range(G):
        m[g, :, :, g] = 1.0
    mask_dev = m.reshape(P, 2 * G).astype(bf16)
    return {
        "x": x_dev,
        "w_se": wse_dev,
        "w_conv": wconv_dev,
        "w_se1": w1,
        "bd_mask": mask_dev,
    }


def postprocess(raw_out):
    """raw_out: [4, 2, 128, 3136] bf16 -> [4, 256, 56, 56] f32."""
    return np.asarray(raw_out, dtype=np.float32).reshape(B_PER_CORE, O, 56, 56)


_NC_CACHE = None
LAST_RESULT = None


def kernel(x, w_conv, w_se1, w_se_out):
    global _NC_CACHE
    from concourse.bass_utils import run_bass_kernel_spmd

    if _NC_CACHE is None:
        _NC_CACHE = build_nc()
    nc = _NC_CACHE

    B = x.shape[0]
    in_maps = []
    for i in range(N_CORES):
        shard = x[i * B_PER_CORE : (i + 1) * B_PER_CORE]
        in_maps.append(prep_core_inputs(shard, w_conv, w_se1, w_se_out))

    global LAST_RESULT
    res = run_bass_kernel_spmd(nc, in_maps, core_ids=list(range(N_CORES)))
    LAST_RESULT = res
    out = np.concatenate(
        [postprocess(res.results[i]["out"]) for i in range(N_CORES)], axis=0
    )
    assert out.shape == (B, O, 56, 56)
    return out


# revision 6
# speedup vs baseline: 1.1805x; 1.1805x over previous
"""AdaptConv2d Trainium2 kernel v3: per-sample adapted 1x1 conv (SE-modulated).

Reference computation (B=32, C=O=256, H=W=56, HID=16):
    pooled = mean(x, (2,3))                      [B, C]
    hid    = silu(pooled @ w_se1.T)              [B, 16]
    adapt  = (hid @ w_se_out.T).reshape(B,O,C)   [B, O, C]
    out[b] = (w_conv + adapt[b]) @ x[b]          [B, O, H*W]

Distribution: data-parallel over batch, 4 samples per core on 8 cores.

v3 structure (clean phases, engine streams kept in dependency order):
  - all load DMAs dispatched up front: x halves (quarters for the last
    two samples) alternating both HWDGE rings, w_se column chunks after
    x so pooling finishes as early as possible.
  - pools on ACT/DVE alternating by arrival; junk matmuls per arriving
    chunk keep the PE HAM clock warm until the adapt matmul.
  - ONE adapt pass for all 4 samples: bd [128, 32] (cols b*8+g), 16
    psum tiles of 512 streaed against arriving w_se chunks, evictions
    alternate DVE/ACT into adapt_sb [32, 8192] (row b*8+g).
  - reshape to per-sample lhsT via DRAM bounce: 1 store + 2 strided
    loads per sample (alternating rings), then w_conv added on DVE.
  - main GEMM per sample in bf16, psum groups of 3, evictions
    alternate DVE/ACT, output stores all dispatched from the sync ring.
"""

import numpy as np

B_PER_CORE = 4
N_CORES = 8
C = 256
O = 256
HW = 3136
HWH = HW // 2  # 1568
HID = 16
P = 128
G = 8
NT = 448  # free-dim tile of the main GEMM (7 * 448 = 3136)
NN = HW // NT
ADN = C * O // G  # 8192 streaming columns for adapt
ADT = ADN // 512  # 16 psum tiles
WSE_CHUNK = 2048
JUNK_HALF = 4  # warmup matmuls per arriving x half (samples 0,1)
JUNK_Q = 2  # warmup matmuls per arriving x quarter (samples 2,3)


def build_nc():
    from concourse import bacc, tile, mybir

    f32 = mybir.dt.float32
    bf16 = mybir.dt.bfloat16

    nc = bacc.Bacc("TRN2", target_bir_lowering=False, debug=False)

    x_d = nc.dram_tensor("x", [B_PER_CORE, P, 2 * HW], bf16, kind="ExternalInput")
    wse_d = nc.dram_tensor("w_se", [P, ADN], bf16, kind="ExternalInput")
    wconv_d = nc.dram_tensor("w_conv", [P, 2 * O], bf16, kind="ExternalInput")
    wse1_d = nc.dram_tensor("w_se1", [P, 2 * P], f32, kind="ExternalInput")
    mask_d = nc.dram_tensor(
        "bd_mask", [P, B_PER_CORE * G], bf16, kind="ExternalInput"
    )
    adsc_d = nc.dram_tensor("adsc", [B_PER_CORE * G, ADN], bf16, kind="Internal")
    out_d = nc.dram_tensor("out", [B_PER_CORE, 2, P, HW], bf16, kind="ExternalOutput")

    with tile.TileContext(nc) as tc:
        with (
            tc.tile_pool(name="xp", bufs=B_PER_CORE) as xp,
            tc.tile_pool(name="wsep", bufs=1) as wsep,
            tc.tile_pool(name="consts", bufs=1) as consts,
            tc.tile_pool(name="adp", bufs=1) as adp,
            tc.tile_pool(name="wbp", bufs=1) as wbp,
            tc.tile_pool(name="small", bufs=1) as small,
            tc.tile_pool(name="scratch", bufs=2) as scratchp,
            tc.tile_pool(name="stage", bufs=3) as stagep,
            tc.tile_pool(name="ps_ad", bufs=3, space="PSUM") as ps_ad,
            tc.tile_pool(name="ps_z", bufs=1, space="PSUM") as ps_z,
            tc.tile_pool(name="ps_mm", bufs=4, space="PSUM") as ps_mm,
        ):
            # ---- consts (small, ahead of x on both rings) ----
            wconv_sb = consts.tile([P, 2 * O], bf16, tag="wconv")
            nc.sync.dma_start(out=wconv_sb[:], in_=wconv_d.ap()[:])
            wse1_sb = consts.tile([P, 2 * P], f32, tag="wse1")
            nc.scalar.dma_start(out=wse1_sb[:], in_=wse1_d.ap()[:])
            mask_sb = consts.tile([P, B_PER_CORE * G], bf16, tag="mask")
            nc.scalar.dma_start(out=mask_sb[:], in_=mask_d.ap()[:])

            # prepay the sigmoid LUT load while DMAs stream
            lutw = small.tile([P, 1], f32, tag="lutw")
            nc.scalar.activation(
                lutw[:], wse1_sb[:, 0:1], mybir.ActivationFunctionType.Sigmoid
            )

            x_tiles = [
                xp.tile([P, 2 * HW], bf16, tag="x", name=f"xt{b}")
                for b in range(B_PER_CORE)
            ]
            wse_sb = wsep.tile([P, ADN], bf16)

            # ---- all load dispatches up front ----
            # chunk list: (b, offset, size); halves for b0/b1, quarters b2/b3
            chunks = []
            for b in (0, 1):
                for k in range(2):
                    chunks.append((b, k * HW, HW))
            for b in (2, 3):
                for q in range(4):
                    chunks.append((b, q * HWH, HWH))
            for i, (b, off, sz) in enumerate(chunks):
                eng = nc.sync if i % 2 == 0 else nc.scalar
                eng.dma_start(
                    out=x_tiles[b][:, off : off + sz],
                    in_=x_d.ap()[b][:, off : off + sz],
                )
            for ci in range(ADN // WSE_CHUNK):
                eng = nc.sync if ci % 2 == 0 else nc.scalar
                eng.dma_start(
                    out=wse_sb[:, ci * WSE_CHUNK : (ci + 1) * WSE_CHUNK],
                    in_=wse_d.ap()[:, ci * WSE_CHUNK : (ci + 1) * WSE_CHUNK],
                )

            # ---- junk warmup + pooling per arriving chunk ----
            # pooled_p[k][:, col] partial sums; cols laid out per chunk:
            #   b0/b1 halves: col = b (one col per k-half)
            #   b2/b3 quarters: col = 4 + b_local*2... handled via table
            pooled_p = [
                small.tile([P, 8], f32, tag=f"poolp{k}", name=f"poolp{k}")
                for k in range(2)
            ]
            # per-sample column lists per k: samples 0,1 -> 1 partial; 2,3 -> 2
            pool_cols = {0: [0], 1: [1], 2: [2, 3], 3: [4, 5]}

            ev = 0
            pe = 0  # pool engine round-robin
            njunk = 0
            for i, (b, off, sz) in enumerate(chunks):
                k = off // HW
                j = (off % HW) // HWH
                nj = JUNK_HALF if sz == HW else JUNK_Q
                if i >= len(chunks) - 2:
                    nj = 1
                for _ in range(nj):
                    junk_ps = ps_mm.tile(
                        [P, NT], f32, tag="mmps", name=f"junk{njunk}"
                    )
                    nc.tensor.matmul(
                        junk_ps[:],
                        lhsT=x_tiles[b][:, off : off + P],
                        rhs=x_tiles[b][:, off : off + NT],
                        start=True,
                        stop=True,
                    )
                    njunk += 1
                col = pool_cols[b][j] if sz == HWH else pool_cols[b][0]
                if pe % 2 == 0:
                    sca = scratchp.tile(
                        [P, sz], bf16, tag="poolscratch", name=f"psc{i}"
                    )
                    nc.scalar.activation(
                        out=sca[:],
                        in_=x_tiles[b][:, off : off + sz],
                        func=mybir.ActivationFunctionType.Copy,
                        accum_out=pooled_p[k][:, col : col + 1],
                    )
                else:
                    nc.vector.tensor_reduce(
                        out=pooled_p[k][:, col : col + 1],
                        in_=x_tiles[b][:, off : off + sz],
                        axis=mybir.AxisListType.X,
                        op=mybir.AluOpType.add,
                    )
                pe += 1

            # ---- combine partials -> pooled[k] [128, B] ----
            pooled = [
                small.tile([P, B_PER_CORE], f32, tag=f"pool{k}", name=f"pool{k}")
                for k in range(2)
            ]
            for k in range(2):
                nc.vector.tensor_copy(
                    out=pooled[k][:, 0:2], in_=pooled_p[k][:, 0:2]
                )
                nc.vector.tensor_tensor(
                    out=pooled[k][:, 2:4],
                    in0=pooled_p[k][:, 2:6:2],
                    in1=pooled_p[k][:, 3:7:2],
                    op=mybir.AluOpType.add,
                )

            # ---- SE chain: z -> silu -> bd [128, B*G] (col = b*8+g') ----
            z_ps = ps_z.tile([P, B_PER_CORE], f32, tag="z")
            for k in range(2):
                nc.tensor.matmul(
                    z_ps[:],
                    lhsT=wse1_sb[:, k * P : (k + 1) * P],
                    rhs=pooled[k][:],
                    start=(k == 0),
                    stop=(k == 1),
                )
            sig = small.tile([P, B_PER_CORE], f32, tag="sig")
            nc.scalar.activation(
                sig[:], z_ps[:], mybir.ActivationFunctionType.Sigmoid
            )
            zs = small.tile([P, B_PER_CORE], f32, tag="zs")
            nc.vector.tensor_tensor(
                out=zs[:], in0=sig[:], in1=z_ps[:], op=mybir.AluOpType.mult
            )
            bd = small.tile([P, B_PER_CORE * G], bf16, tag="bd")
            nc.vector.tensor_tensor(
                out=bd[:].rearrange("p (b g) -> p b g", b=B_PER_CORE, g=G),
                in0=zs[:].unsqueeze(2).broadcast_to([P, B_PER_CORE, G]),
                in1=mask_sb[:].rearrange("p (b g) -> p b g", b=B_PER_CORE, g=G),
                op=mybir.AluOpType.mult,
            )

            # ---- adapt matmul: 16 psum tiles of 512 cols ----
            adapt_sb = adp.tile([B_PER_CORE * G, ADN], bf16, tag="adapt")
            for t in range(ADT):
                ap_ps = ps_ad.tile(
                    [B_PER_CORE * G, 512], f32, tag="adps", name=f"adps{t}", bufs=3
                )
                nc.tensor.matmul(
                    ap_ps[:],
                    lhsT=bd[:],
                    rhs=wse_sb[:, t * 512 : (t + 1) * 512],
                    start=True,
                    stop=True,
                )
                if ev % 2 == 0:
                    nc.vector.tensor_copy(
                        out=adapt_sb[:, t * 512 : (t + 1) * 512], in_=ap_ps[:]
                    )
                else:
                    nc.scalar.copy(
                        out=adapt_sb[:, t * 512 : (t + 1) * 512], in_=ap_ps[:]
                    )
                ev += 1

            # ---- reshape through DRAM + add w_conv -> wb[b] [128, 2*O] ----
            nc.sync.dma_start(out=adsc_d.ap()[:], in_=adapt_sb[:])
            wbs = []
            for b in range(B_PER_CORE):
                wb = wbp.tile([P, 2 * O], bf16, tag=f"wb{b}", name=f"wb{b}")
                wbs.append(wb)
                for k in range(2):
                    r0 = b * 8 + k * 4
                    src = adsc_d.ap()[r0 : r0 + 4, :].rearrange(
                        "gl (cl o) -> gl cl o", cl=32, o=O
                    )
                    eng = nc.sync if (b * 2 + k) % 2 == 0 else nc.scalar
                    eng.dma_start(out=wb[:, k * O : (k + 1) * O], in_=src)
                nc.vector.tensor_tensor(
                    out=wb[:], in0=wb[:], in1=wconv_sb[:], op=mybir.AluOpType.add
                )

            # ---- main GEMM ----
            for b in range(B_PER_CORE):
                for oc in range(2):
                    stage = stagep.tile(
                        [P, HW], bf16, tag="stage", name=f"st{b}{oc}"
                    )
                    for grp_n in ((0, 1, 2), (3, 4, 5), (6,)):
                        pss = [
                            ps_mm.tile(
                                [P, NT], f32, tag="mmps", name=f"ps{b}{oc}{n}"
                            )
                            for n in grp_n
                        ]
                        for k in range(2):
                            w_slice = wbs[b][:, k * O + oc * P : k * O + oc * P + P]
                            for i, n in enumerate(grp_n):
                                nc.tensor.matmul(
                                    pss[i][:],
                                    lhsT=w_slice,
                                    rhs=x_tiles[b][
                                        :, k * HW + n * NT : k * HW + (n + 1) * NT
                                    ],
                                    start=(k == 0),
                                    stop=(k == 1),
                                )
                        for i, n in enumerate(grp_n):
                            if ev % 2 == 0:
                                nc.vector.tensor_copy(
                                    out=stage[:, n * NT : (n + 1) * NT],
                                    in_=pss[i][:],
                                )
                            else:
                                nc.scalar.copy(
                                    out=stage[:, n * NT : (n + 1) * NT],
                                    in_=pss[i][:],
                                )
                            ev += 1
                    nc.sync.dma_start(out=out_d.ap()[b, oc], in_=stage[:])

    nc.compile()
    return nc


def prep_core_inputs(x_shard, w_conv, w_se1, w_se_out):
    """Host-side layout prep for one core. x_shard: [4, 256, 56, 56] f32."""
    import ml_dtypes

    bf16 = ml_dtypes.bfloat16
    b = x_shard.shape[0]
    # x: [b, 128, 2*3136], c = k*128 + p, free = k*3136 + hw
    xr = x_shard.reshape(b, 2, P, HW).transpose(0, 2, 1, 3).reshape(b, P, 2 * HW)
    x_dev = np.ascontiguousarray(xr).astype(bf16)
    # w_se: [(g,h), n] with flat = c*256 + o = g*8192 + n, n = cl*256 + o
    w_r = w_se_out.reshape(O, C, HID).transpose(1, 0, 2)  # [c, o, h]
    w_r = w_r.reshape(G, ADN, HID).transpose(0, 2, 1).reshape(P, ADN)
    wse_dev = np.ascontiguousarray(w_r).astype(bf16)
    # w_conv in per-sample lhsT layout: [c_p, k*256 + o] = w_conv[o, k*128 + c_p]
    wc = w_conv[:, :, 0, 0]  # [O, C]
    wconv_dev = np.ascontiguousarray(
        wc.T.reshape(2, P, O).transpose(1, 0, 2).reshape(P, 2 * O)
    ).astype(bf16)
    # w_se1 replicated for the G h-groups:
    # [p, k*128 + (g*16+h)] = w_se1[h, k*128+p] / 3136
    w1 = (w_se1.T / float(HW)).reshape(2, P, HID)  # [k, p, h]
    w1 = np.broadcast_to(w1[:, :, None, :], (2, P, G, HID)).reshape(2, P, P)
    w1 = np.ascontiguousarray(w1.transpose(1, 0, 2).reshape(P, 2 * P)).astype(
        np.float32
    )
    # bd mask: [(g,h), b*8 + g'] = 1 if g == g'
    m = np.zeros((G, HID, B_PER_CORE, G), np.float32)
    for g in range(G):
        m[g, :, :, g] = 1.0
    mask_dev = m.reshape(P, B_PER_CORE * G).astype(bf16)
    return {
        "x": x_dev,
        "w_se": wse_dev,
        "w_conv": wconv_dev,
        "w_se1": w1,
        "bd_mask": mask_dev,
    }


def postprocess(raw_out):
    """raw_out: [4, 2, 128, 3136] bf16 -> [4, 256, 56, 56] f32."""
    return np.asarray(raw_out, dtype=np.float32).reshape(B_PER_CORE, O, 56, 56)


_NC_CACHE = None
LAST_RESULT = None


def kernel(x, w_conv, w_se1, w_se_out):
    global _NC_CACHE
    from concourse.bass_utils import run_bass_kernel_spmd

    if _NC_CACHE is None:
        _NC_CACHE = build_nc()
    nc = _NC_CACHE

    B = x.shape[0]
    in_maps = []
    for i in range(N_CORES):
        shard = x[i * B_PER_CORE : (i + 1) * B_PER_CORE]
        in_maps.append(prep_core_inputs(shard, w_conv, w_se1, w_se_out))

    global LAST_RESULT
    res = run_bass_kernel_spmd(nc, in_maps, core_ids=list(range(N_CORES)))
    LAST_RESULT = res
    out = np.concatenate(
        [postprocess(res.results[i]["out"]) for i in range(N_CORES)], axis=0
    )
    assert out.shape == (B, O, 56, 56)
    return out


# revision 10
# speedup vs baseline: 1.1900x; 1.0081x over previous
"""AdaptConv2d Trainium2 kernel v3: per-sample adapted 1x1 conv (SE-modulated).

Reference computation (B=32, C=O=256, H=W=56, HID=16):
    pooled = mean(x, (2,3))                      [B, C]
    hid    = silu(pooled @ w_se1.T)              [B, 16]
    adapt  = (hid @ w_se_out.T).reshape(B,O,C)   [B, O, C]
    out[b] = (w_conv + adapt[b]) @ x[b]          [B, O, H*W]

Distribution: data-parallel over batch, 4 samples per core on 8 cores.

v3 structure (clean phases, engine streams kept in dependency order):
  - all load DMAs dispatched up front: x halves (quarters for the last
    two samples) alternating both HWDGE rings, w_se column chunks after
    x so pooling finishes as early as possible.
  - pools on ACT/DVE alternating by arrival; junk matmuls per arriving
    chunk keep the PE HAM clock warm until the adapt matmul.
  - ONE adapt pass for all 4 samples: bd [128, 32] (cols b*8+g), 16
    psum tiles of 512 streaed against arriving w_se chunks, evictions
    alternate DVE/ACT into adapt_sb [32, 8192] (row b*8+g).
  - reshape to per-sample lhsT via DRAM bounce: 1 store + 2 strided
    loads per sample (alternating rings), then w_conv added on DVE.
  - main GEMM per sample in bf16, psum groups of 3, evictions
    alternate DVE/ACT, output stores all dispatched from the sync ring.
"""

import numpy as np

B_PER_CORE = 4
N_CORES = 8
C = 256
O = 256
HW = 3136
HWH = HW // 2  # 1568
HID = 16
P = 128
G = 8
NT = 448  # free-dim tile of the main GEMM (7 * 448 = 3136)
NN = HW // NT
ADN = C * O // G  # 8192 streaming columns for adapt
ADT = ADN // 512  # 16 psum tiles
WSE_CHUNK = 2048
JUNK_BLOCK = 56  # back-to-back warmup matmuls on x0 data before the SE chain


def build_nc():
    from concourse import bacc, tile, mybir

    f32 = mybir.dt.float32
    bf16 = mybir.dt.bfloat16

    nc = bacc.Bacc("TRN2", target_bir_lowering=False, debug=False)

    x_d = nc.dram_tensor("x", [B_PER_CORE, P, 2 * HW], bf16, kind="ExternalInput")
    wse_d = nc.dram_tensor("w_se", [P, ADN], bf16, kind="ExternalInput")
    wconv_d = nc.dram_tensor("w_conv", [P, 2 * O], bf16, kind="ExternalInput")
    wse1_d = nc.dram_tensor("w_se1", [P, 2 * P], f32, kind="ExternalInput")
    mask_d = nc.dram_tensor(
        "bd_mask", [P, B_PER_CORE * G], bf16, kind="ExternalInput"
    )
    adsc_d = nc.dram_tensor("adsc", [B_PER_CORE * G, ADN], bf16, kind="Internal")
    out_d = nc.dram_tensor("out", [B_PER_CORE, 2, P, HW], bf16, kind="ExternalOutput")

    with tile.TileContext(nc) as tc:
        with (
            tc.tile_pool(name="xp", bufs=B_PER_CORE) as xp,
            tc.tile_pool(name="wsep", bufs=1) as wsep,
            tc.tile_pool(name="consts", bufs=1) as consts,
            tc.tile_pool(name="adp", bufs=1) as adp,
            tc.tile_pool(name="wbp", bufs=1) as wbp,
            tc.tile_pool(name="small", bufs=1) as small,
            tc.tile_pool(name="scratch", bufs=2) as scratchp,
            tc.tile_pool(name="stage", bufs=3) as stagep,
            tc.tile_pool(name="ps_ad", bufs=3, space="PSUM") as ps_ad,
            tc.tile_pool(name="ps_z", bufs=1, space="PSUM") as ps_z,
            tc.tile_pool(name="ps_mm", bufs=4, space="PSUM") as ps_mm,
        ):
            # ---- consts (small, ahead of x on both rings) ----
            wconv_sb = consts.tile([P, 2 * O], bf16, tag="wconv")
            nc.sync.dma_start(out=wconv_sb[:], in_=wconv_d.ap()[:])
            wse1_sb = consts.tile([P, 2 * P], f32, tag="wse1")
            nc.scalar.dma_start(out=wse1_sb[:], in_=wse1_d.ap()[:])
            mask_sb = consts.tile([P, B_PER_CORE * G], bf16, tag="mask")
            nc.scalar.dma_start(out=mask_sb[:], in_=mask_d.ap()[:])

            # prepay the sigmoid LUT load while DMAs stream
            lutw = small.tile([P, 1], f32, tag="lutw")
            nc.scalar.activation(
                lutw[:], wse1_sb[:, 0:1], mybir.ActivationFunctionType.Sigmoid
            )

            x_tiles = [
                xp.tile([P, 2 * HW], bf16, tag="x", name=f"xt{b}")
                for b in range(B_PER_CORE)
            ]
            wse_sb = wsep.tile([P, ADN], bf16)

            # ---- all load dispatches up front ----
            # x halves: h0 (k=0) on sync, h1 (k=1) on scalar, sample-major;
            # wse column chunks LAST (they stream against the adapt matmul)
            for b in range(B_PER_CORE):
                nc.sync.dma_start(
                    out=x_tiles[b][:, 0:HW], in_=x_d.ap()[b][:, 0:HW]
                )
                nc.scalar.dma_start(
                    out=x_tiles[b][:, HW : 2 * HW], in_=x_d.ap()[b][:, HW : 2 * HW]
                )
            for ci in range(ADN // WSE_CHUNK):
                eng = nc.sync if ci % 2 == 0 else nc.scalar
                eng.dma_start(
                    out=wse_sb[:, ci * WSE_CHUNK : (ci + 1) * WSE_CHUNK],
                    in_=wse_d.ap()[:, ci * WSE_CHUNK : (ci + 1) * WSE_CHUNK],
                )

            # ---- junk warmup block: back-to-back matmuls on x0 data ----
            for j in range(JUNK_BLOCK):
                junk_ps = ps_mm.tile([P, NT], f32, tag="mmps", name=f"junk{j}")
                nc.tensor.matmul(
                    junk_ps[:],
                    lhsT=x_tiles[0][:, 0:P],
                    rhs=x_tiles[0][:, 0:NT],
                    start=True,
                    stop=True,
                )

            # ---- pooling, in arrival order ----
            # pooled_p[k][:, col]: b0..b2 -> col b (full half); b3 -> cols 3,4
            # (two slices per half, split across ACT and DVE)
            pooled_p = [
                small.tile([P, 8], f32, tag=f"poolp{k}", name=f"poolp{k}")
                for k in range(2)
            ]

            def pool_one(eng_act, b, k, off, sz, col, i):
                if eng_act:
                    sca = scratchp.tile(
                        [P, sz], bf16, tag="poolscratch", name=f"psc{i}"
                    )
                    nc.scalar.activation(
                        out=sca[:],
                        in_=x_tiles[b][:, off : off + sz],
                        func=mybir.ActivationFunctionType.Copy,
                        accum_out=pooled_p[k][:, col : col + 1],
                    )
                else:
                    nc.vector.tensor_reduce(
                        out=pooled_p[k][:, col : col + 1],
                        in_=x_tiles[b][:, off : off + sz],
                        axis=mybir.AxisListType.X,
                        op=mybir.AluOpType.add,
                    )

            for b in (0, 1, 2):
                pool_one(True, b, 0, 0, HW, b, 2 * b)
                pool_one(False, b, 1, HW, HW, b, 2 * b + 1)
            # b3: slice each half in two, ACT + DVE per half
            pool_one(True, 3, 0, 0, HWH, 3, 6)
            pool_one(False, 3, 0, HWH, HWH, 4, 7)
            pool_one(True, 3, 1, HW, HWH, 3, 8)
            pool_one(False, 3, 1, HW + HWH, HWH, 4, 9)

            # ---- combine partials -> pooled[k] [128, B] ----
            pooled = [
                small.tile([P, B_PER_CORE], f32, tag=f"pool{k}", name=f"pool{k}")
                for k in range(2)
            ]
            ev = 0
            for k in range(2):
                nc.vector.tensor_copy(
                    out=pooled[k][:, 0:3], in_=pooled_p[k][:, 0:3]
                )
                nc.vector.tensor_tensor(
                    out=pooled[k][:, 3:4],
                    in0=pooled_p[k][:, 3:4],
                    in1=pooled_p[k][:, 4:5],
                    op=mybir.AluOpType.add,
                )

            # ---- SE chain: z -> silu -> bd [128, B*G] (col = b*8+g') ----
            z_ps = ps_z.tile([P, B_PER_CORE], f32, tag="z")
            for k in range(2):
                nc.tensor.matmul(
                    z_ps[:],
                    lhsT=wse1_sb[:, k * P : (k + 1) * P],
                    rhs=pooled[k][:],
                    start=(k == 0),
                    stop=(k == 1),
                )
            sig = small.tile([P, B_PER_CORE], f32, tag="sig")
            nc.scalar.activation(
                sig[:], z_ps[:], mybir.ActivationFunctionType.Sigmoid
            )
            zs = small.tile([P, B_PER_CORE], f32, tag="zs")
            nc.vector.tensor_tensor(
                out=zs[:], in0=sig[:], in1=z_ps[:], op=mybir.AluOpType.mult
            )
            bd = small.tile([P, B_PER_CORE * G], bf16, tag="bd")
            nc.vector.tensor_tensor(
                out=bd[:].rearrange("p (b g) -> p b g", b=B_PER_CORE, g=G),
                in0=zs[:].unsqueeze(2).broadcast_to([P, B_PER_CORE, G]),
                in1=mask_sb[:].rearrange("p (b g) -> p b g", b=B_PER_CORE, g=G),
                op=mybir.AluOpType.mult,
            )

            # ---- adapt matmul: 16 psum tiles of 512 cols; store to DRAM in
            # 4 column chunks as evictions complete ----
            adapt_sb = adp.tile([B_PER_CORE * G, ADN], bf16, tag="adapt")
            for t in range(ADT):
                ap_ps = ps_ad.tile(
                    [B_PER_CORE * G, 512], f32, tag="adps", name=f"adps{t}", bufs=3
                )
                nc.tensor.matmul(
                    ap_ps[:],
                    lhsT=bd[:],
                    rhs=wse_sb[:, t * 512 : (t + 1) * 512],
                    start=True,
                    stop=True,
                )
                if ev % 2 == 0:
                    nc.vector.tensor_copy(
                        out=adapt_sb[:, t * 512 : (t + 1) * 512], in_=ap_ps[:]
                    )
                else:
                    nc.scalar.copy(
                        out=adapt_sb[:, t * 512 : (t + 1) * 512], in_=ap_ps[:]
                    )
                ev += 1
                if t % 4 == 3:
                    c0 = (t - 3) * 512
                    nc.sync.dma_start(
                        out=adsc_d.ap()[:, c0 : c0 + 2048],
                        in_=adapt_sb[:, c0 : c0 + 2048],
                    )

            # ---- reshape through DRAM + add w_conv -> wb[b] [128, 2*O] ----
            wbs = []
            for b in range(B_PER_CORE):
                wb = wbp.tile([P, 2 * O], bf16, tag=f"wb{b}", name=f"wb{b}")
                wbs.append(wb)
                for k in range(2):
                    r0 = b * 8 + k * 4
                    src = adsc_d.ap()[r0 : r0 + 4, :].rearrange(
                        "gl (cl o) -> gl cl o", cl=32, o=O
                    )
                    eng = nc.sync if (b * 2 + k) % 2 == 0 else nc.scalar
                    eng.dma_start(out=wb[:, k * O : (k + 1) * O], in_=src)
                    nc.vector.tensor_tensor(
                        out=wb[:, k * O : (k + 1) * O],
                        in0=wb[:, k * O : (k + 1) * O],
                        in1=wconv_sb[:, k * O : (k + 1) * O],
                        op=mybir.AluOpType.add,
                    )

            # ---- main GEMM ----
            for b in range(B_PER_CORE):
                for oc in range(2):
                    stage = stagep.tile(
                        [P, HW], bf16, tag="stage", name=f"st{b}{oc}"
                    )
                    for grp_n in ((0, 1, 2), (3, 4, 5), (6,)):
                        pss = [
                            ps_mm.tile(
                                [P, NT], f32, tag="mmps", name=f"ps{b}{oc}{n}"
                            )
                            for n in grp_n
                        ]
                        for k in range(2):
                            w_slice = wbs[b][:, k * O + oc * P : k * O + oc * P + P]
                            for i, n in enumerate(grp_n):
                                nc.tensor.matmul(
                                    pss[i][:],
                                    lhsT=w_slice,
                                    rhs=x_tiles[b][
                                        :, k * HW + n * NT : k * HW + (n + 1) * NT
                                    ],
                                    start=(k == 0),
                                    stop=(k == 1),
                                )
                        for i, n in enumerate(grp_n):
                            if ev % 2 == 0:
                                nc.vector.tensor_copy(
                                    out=stage[:, n * NT : (n + 1) * NT],
                                    in_=pss[i][:],
                                )
                            else:
                                nc.scalar.copy(
                                    out=stage[:, n * NT : (n + 1) * NT],
                                    in_=pss[i][:],
                                )
                            ev += 1
                    oeng = nc.sync if (b * 2 + oc) % 2 == 0 else nc.scalar
                    oeng.dma_start(out=out_d.ap()[b, oc], in_=stage[:])

    nc.compile()
    return nc


def prep_core_inputs(x_shard, w_conv, w_se1, w_se_out):
    """Host-side layout prep for one core. x_shard: [4, 256, 56, 56] f32."""
    import ml_dtypes

    bf16 = ml_dtypes.bfloat16
    b = x_shard.shape[0]
    # x: [b, 128, 2*3136], c = k*128 + p, free = k*3136 + hw
    xr = x_shard.reshape(b, 2, P, HW).transpose(0, 2, 1, 3).reshape(b, P, 2 * HW)
    x_dev = np.ascontiguousarray(xr).astype(bf16)
    # w_se: [(g,h), n] with flat = c*256 + o = g*8192 + n, n = cl*256 + o
    w_r = w_se_out.reshape(O, C, HID).transpose(1, 0, 2)  # [c, o, h]
    w_r = w_r.reshape(G, ADN, HID).transpose(0, 2, 1).reshape(P, ADN)
    wse_dev = np.ascontiguousarray(w_r).astype(bf16)
    # w_conv in per-sample lhsT layout: [c_p, k*256 + o] = w_conv[o, k*128 + c_p]
    wc = w_conv[:, :, 0, 0]  # [O, C]
    wconv_dev = np.ascontiguousarray(
        wc.T.reshape(2, P, O).transpose(1, 0, 2).reshape(P, 2 * O)
    ).astype(bf16)
    # w_se1 replicated for the G h-groups:
    # [p, k*128 + (g*16+h)] = w_se1[h, k*128+p] / 3136
    w1 = (w_se1.T / float(HW)).reshape(2, P, HID)  # [k, p, h]
    w1 = np.broadcast_to(w1[:, :, None, :], (2, P, G, HID)).reshape(2, P, P)
    w1 = np.ascontiguousarray(w1.transpose(1, 0, 2).reshape(P, 2 * P)).astype(
        np.float32
    )
    # bd mask: [(g,h), b*8 + g'] = 1 if g == g'
    m = np.zeros((G, HID, B_PER_CORE, G), np.float32)
    for g in range(G):
        m[g, :, :, g] = 1.0
    mask_dev = m.reshape(P, B_PER_CORE * G).astype(bf16)
    return {
        "x": x_dev,
        "w_se": wse_dev,
        "w_conv": wconv_dev,
        "w_se1": w1,
        "bd_mask": mask_dev,
    }


def postprocess(raw_out):
    """raw_out: [4, 2, 128, 3136] bf16 -> [4, 256, 56, 56] f32."""
    return np.asarray(raw_out, dtype=np.float32).reshape(B_PER_CORE, O, 56, 56)


_NC_CACHE = None
LAST_RESULT = None


def kernel(x, w_conv, w_se1, w_se_out):
    global _NC_CACHE
    from concourse.bass_utils import run_bass_kernel_spmd

    if _NC_CACHE is None:
        _NC_CACHE = build_nc()
    nc = _NC_CACHE

    B = x.shape[0]
    in_maps = []
    for i in range(N_CORES):
        shard = x[i * B_PER_CORE : (i + 1) * B_PER_CORE]
        in_maps.append(prep_core_inputs(shard, w_conv, w_se1, w_se_out))

    global LAST_RESULT
    res = run_bass_kernel_spmd(nc, in_maps, core_ids=list(range(N_CORES)))
    LAST_RESULT = res
    out = np.concatenate(
        [postprocess(res.results[i]["out"]) for i in range(N_CORES)], axis=0
    )
    assert out.shape == (B, O, 56, 56)
    return out


# revision 13
# speedup vs baseline: 1.2848x; 1.0797x over previous
"""AdaptConv2d Trainium2 kernel v3: per-sample adapted 1x1 conv (SE-modulated).

Reference computation (B=32, C=O=256, H=W=56, HID=16):
    pooled = mean(x, (2,3))                      [B, C]
    hid    = silu(pooled @ w_se1.T)              [B, 16]
    adapt  = (hid @ w_se_out.T).reshape(B,O,C)   [B, O, C]
    out[b] = (w_conv + adapt[b]) @ x[b]          [B, O, H*W]

Distribution: data-parallel over batch, 4 samples per core on 8 cores.

v3 structure (clean phases, engine streams kept in dependency order):
  - all load DMAs dispatched up front: x halves (quarters for the last
    two samples) alternating both HWDGE rings, w_se column chunks after
    x so pooling finishes as early as possible.
  - pools on ACT/DVE alternating by arrival; junk matmuls per arriving
    chunk keep the PE HAM clock warm until the adapt matmul.
  - ONE adapt pass for all 4 samples: bd [128, 32] (cols b*8+g), 16
    psum tiles of 512 streaed against arriving w_se chunks, evictions
    alternate DVE/ACT into adapt_sb [32, 8192] (row b*8+g).
  - reshape to per-sample lhsT via DRAM bounce: 1 store + 2 strided
    loads per sample (alternating rings), then w_conv added on DVE.
  - main GEMM per sample in bf16, psum groups of 3, evictions
    alternate DVE/ACT, output stores all dispatched from the sync ring.
"""

import numpy as np

B_PER_CORE = 4
N_CORES = 8
C = 256
O = 256
HW = 3136
HWH = HW // 2  # 1568
HID = 16
P = 128
G = 8
NT = 448  # free-dim tile of the main GEMM (7 * 448 = 3136)
NN = HW // NT
ADN = C * O // G  # 8192 streaming columns for adapt
ADT = ADN // 512  # 16 psum tiles
WSE_CHUNK = 2048
JUNK_BLOCK = 76  # back-to-back warmup matmuls on x0 data before the SE chain


def build_nc():
    from concourse import bacc, tile, mybir

    f32 = mybir.dt.float32
    bf16 = mybir.dt.bfloat16

    nc = bacc.Bacc("TRN2", target_bir_lowering=False, debug=False)

    x_d = nc.dram_tensor("x", [B_PER_CORE, P, 2 * HW], bf16, kind="ExternalInput")
    wse_d = nc.dram_tensor("w_se", [P, ADN], bf16, kind="ExternalInput")
    wconv_d = nc.dram_tensor("w_conv", [P, 2 * O], bf16, kind="ExternalInput")
    wse1_d = nc.dram_tensor("w_se1", [P, 2 * P], f32, kind="ExternalInput")
    mask_d = nc.dram_tensor(
        "bd_mask", [P, B_PER_CORE * G], bf16, kind="ExternalInput"
    )
    adsc_d = nc.dram_tensor("adsc", [B_PER_CORE * G, ADN], bf16, kind="Internal")
    out_d = nc.dram_tensor("out", [B_PER_CORE, 2, P, HW], bf16, kind="ExternalOutput")

    with tile.TileContext(nc) as tc:
        with (
            tc.tile_pool(name="xp", bufs=B_PER_CORE) as xp,
            tc.tile_pool(name="wsep", bufs=1) as wsep,
            tc.tile_pool(name="consts", bufs=1) as consts,
            tc.tile_pool(name="adp", bufs=1) as adp,
            tc.tile_pool(name="wbp", bufs=1) as wbp,
            tc.tile_pool(name="small", bufs=1) as small,
            tc.tile_pool(name="scratch", bufs=2) as scratchp,
            tc.tile_pool(name="stage", bufs=3) as stagep,
            tc.tile_pool(name="ps_ad", bufs=3, space="PSUM") as ps_ad,
            tc.tile_pool(name="ps_z", bufs=1, space="PSUM") as ps_z,
            tc.tile_pool(name="ps_mm", bufs=4, space="PSUM") as ps_mm,
        ):
            # ---- consts (small, ahead of x on both rings) ----
            wconv_sb = consts.tile([P, 2 * O], bf16, tag="wconv")
            nc.sync.dma_start(out=wconv_sb[:], in_=wconv_d.ap()[:])
            wse1_sb = consts.tile([P, 2 * P], f32, tag="wse1")
            nc.scalar.dma_start(out=wse1_sb[:], in_=wse1_d.ap()[:])
            mask_sb = consts.tile([P, B_PER_CORE * G], bf16, tag="mask")
            nc.scalar.dma_start(out=mask_sb[:], in_=mask_d.ap()[:])

            # prepay the sigmoid LUT load while DMAs stream
            lutw = small.tile([P, 1], f32, tag="lutw")
            nc.scalar.activation(
                lutw[:], wse1_sb[:, 0:1], mybir.ActivationFunctionType.Sigmoid
            )

            x_tiles = [
                xp.tile([P, 2 * HW], bf16, tag="x", name=f"xt{b}")
                for b in range(B_PER_CORE)
            ]
            wse_sb = wsep.tile([P, ADN], bf16)

            # ---- all input loads dispatched from SP (sync) ONLY ----
            # The SP engine has no compute duties, so ring-full backpressure
            # on dma_start cannot head-of-line-block pools/evictions (which
            # live on the ACT/DVE streams). One HWDGE ring still saturates
            # HBM: each DMA is split across all 16 SDMA engines.
            # Order: x halves sample-major (pooling path), wse chunks last
            # (they stream against the adapt matmul).
            for b in range(B_PER_CORE):
                nc.sync.dma_start(
                    out=x_tiles[b][:, 0:HW], in_=x_d.ap()[b][:, 0:HW]
                )
                nc.sync.dma_start(
                    out=x_tiles[b][:, HW : 2 * HW], in_=x_d.ap()[b][:, HW : 2 * HW]
                )
            for ci in range(ADN // WSE_CHUNK):
                nc.sync.dma_start(
                    out=wse_sb[:, ci * WSE_CHUNK : (ci + 1) * WSE_CHUNK],
                    in_=wse_d.ap()[:, ci * WSE_CHUNK : (ci + 1) * WSE_CHUNK],
                )

            # ---- junk warmup block: back-to-back matmuls on x0 data ----
            for j in range(JUNK_BLOCK):
                junk_ps = ps_mm.tile([P, NT], f32, tag="mmps", name=f"junk{j}")
                nc.tensor.matmul(
                    junk_ps[:],
                    lhsT=x_tiles[0][:, 0:P],
                    rhs=x_tiles[0][:, 0:NT],
                    start=True,
                    stop=True,
                )

            # ---- pooling, in arrival order ----
            # pooled_p[k][:, col]: b0..b2 -> col b (full half); b3 -> cols 3,4
            # (two slices per half, split across ACT and DVE)
            pooled_p = [
                small.tile([P, 8], f32, tag=f"poolp{k}", name=f"poolp{k}")
                for k in range(2)
            ]

            def pool_one(eng_act, b, k, off, sz, col, i):
                if eng_act:
                    sca = scratchp.tile(
                        [P, sz], bf16, tag="poolscratch", name=f"psc{i}"
                    )
                    nc.scalar.activation(
                        out=sca[:],
                        in_=x_tiles[b][:, off : off + sz],
                        func=mybir.ActivationFunctionType.Copy,
                        accum_out=pooled_p[k][:, col : col + 1],
                    )
                else:
                    nc.vector.tensor_reduce(
                        out=pooled_p[k][:, col : col + 1],
                        in_=x_tiles[b][:, off : off + sz],
                        axis=mybir.AxisListType.X,
                        op=mybir.AluOpType.add,
                    )

            for b in (0, 1, 2):
                pool_one(True, b, 0, 0, HW, b, 2 * b)
                pool_one(False, b, 1, HW, HW, b, 2 * b + 1)
            # b3: slice each half in two, ACT + DVE per half
            pool_one(True, 3, 0, 0, HWH, 3, 6)
            pool_one(False, 3, 0, HWH, HWH, 4, 7)
            pool_one(True, 3, 1, HW, HWH, 3, 8)
            pool_one(False, 3, 1, HW + HWH, HWH, 4, 9)

            # ---- combine partials -> pooled[k] [128, B] ----
            pooled = [
                small.tile([P, B_PER_CORE], f32, tag=f"pool{k}", name=f"pool{k}")
                for k in range(2)
            ]
            ev = 0
            for k in range(2):
                nc.vector.tensor_copy(
                    out=pooled[k][:, 0:3], in_=pooled_p[k][:, 0:3]
                )
                nc.vector.tensor_tensor(
                    out=pooled[k][:, 3:4],
                    in0=pooled_p[k][:, 3:4],
                    in1=pooled_p[k][:, 4:5],
                    op=mybir.AluOpType.add,
                )

            # ---- SE chain: z -> silu -> bd [128, B*G] (col = b*8+g') ----
            z_ps = ps_z.tile([P, B_PER_CORE], f32, tag="z")
            for k in range(2):
                nc.tensor.matmul(
                    z_ps[:],
                    lhsT=wse1_sb[:, k * P : (k + 1) * P],
                    rhs=pooled[k][:],
                    start=(k == 0),
                    stop=(k == 1),
                )
            sig = small.tile([P, B_PER_CORE], f32, tag="sig")
            nc.scalar.activation(
                sig[:], z_ps[:], mybir.ActivationFunctionType.Sigmoid
            )
            zs = small.tile([P, B_PER_CORE], f32, tag="zs")
            nc.vector.tensor_tensor(
                out=zs[:], in0=sig[:], in1=z_ps[:], op=mybir.AluOpType.mult
            )
            bd = small.tile([P, B_PER_CORE * G], bf16, tag="bd")
            nc.vector.tensor_tensor(
                out=bd[:].rearrange("p (b g) -> p b g", b=B_PER_CORE, g=G),
                in0=zs[:].unsqueeze(2).broadcast_to([P, B_PER_CORE, G]),
                in1=mask_sb[:].rearrange("p (b g) -> p b g", b=B_PER_CORE, g=G),
                op=mybir.AluOpType.mult,
            )

            # ---- adapt matmul: 16 psum tiles of 512 cols; store to DRAM in
            # 4 column chunks as evictions complete ----
            adapt_sb = adp.tile([B_PER_CORE * G, ADN], bf16, tag="adapt")
            for t in range(ADT):
                ap_ps = ps_ad.tile(
                    [B_PER_CORE * G, 512], f32, tag="adps", name=f"adps{t}", bufs=3
                )
                nc.tensor.matmul(
                    ap_ps[:],
                    lhsT=bd[:],
                    rhs=wse_sb[:, t * 512 : (t + 1) * 512],
                    start=True,
                    stop=True,
                )
                if ev % 2 == 0:
                    nc.vector.tensor_copy(
                        out=adapt_sb[:, t * 512 : (t + 1) * 512], in_=ap_ps[:]
                    )
                else:
                    nc.scalar.copy(
                        out=adapt_sb[:, t * 512 : (t + 1) * 512], in_=ap_ps[:]
                    )
                ev += 1
                if t % 4 == 3:
                    c0 = (t - 3) * 512
                    nc.sync.dma_start(
                        out=adsc_d.ap()[:, c0 : c0 + 2048],
                        in_=adapt_sb[:, c0 : c0 + 2048],
                    )

            # ---- reshape through DRAM + add w_conv -> wb[b] [128, 2*O] ----
            wbs = []
            for b in range(B_PER_CORE):
                wb = wbp.tile([P, 2 * O], bf16, tag=f"wb{b}", name=f"wb{b}")
                wbs.append(wb)
                for k in range(2):
                    r0 = b * 8 + k * 4
                    src = adsc_d.ap()[r0 : r0 + 4, :].rearrange(
                        "gl (cl o) -> gl cl o", cl=32, o=O
                    )
                    eng = nc.sync if (b * 2 + k) % 2 == 0 else nc.scalar
                    eng.dma_start(out=wb[:, k * O : (k + 1) * O], in_=src)
                    nc.vector.tensor_tensor(
                        out=wb[:, k * O : (k + 1) * O],
                        in0=wb[:, k * O : (k + 1) * O],
                        in1=wconv_sb[:, k * O : (k + 1) * O],
                        op=mybir.AluOpType.add,
                    )

            # ---- main GEMM ----
            for b in range(B_PER_CORE):
                for oc in range(2):
                    stage = stagep.tile(
                        [P, HW], bf16, tag="stage", name=f"st{b}{oc}"
                    )
                    for grp_n in ((0, 1, 2), (3, 4, 5), (6,)):
                        pss = [
                            ps_mm.tile(
                                [P, NT], f32, tag="mmps", name=f"ps{b}{oc}{n}"
                            )
                            for n in grp_n
                        ]
                        for k in range(2):
                            w_slice = wbs[b][:, k * O + oc * P : k * O + oc * P + P]
                            for i, n in enumerate(grp_n):
                                nc.tensor.matmul(
                                    pss[i][:],
                                    lhsT=w_slice,
                                    rhs=x_tiles[b][
                                        :, k * HW + n * NT : k * HW + (n + 1) * NT
                                    ],
                                    start=(k == 0),
                                    stop=(k == 1),
                                )
                        for i, n in enumerate(grp_n):
                            if ev % 2 == 0:
                                nc.vector.tensor_copy(
                                    out=stage[:, n * NT : (n + 1) * NT],
                                    in_=pss[i][:],
                                )
                            else:
                                nc.scalar.copy(
                                    out=stage[:, n * NT : (n + 1) * NT],
                                    in_=pss[i][:],
                                )
                            ev += 1
                    nc.sync.dma_start(out=out_d.ap()[b, oc], in_=stage[:])

    nc.compile()
    return nc


def prep_core_inputs(x_shard, w_conv, w_se1, w_se_out):
    """Host-side layout prep for one core. x_shard: [4, 256, 56, 56] f32."""
    import ml_dtypes

    bf16 = ml_dtypes.bfloat16
    b = x_shard.shape[0]
    # x: [b, 128, 2*3136], c = k*128 + p, free = k*3136 + hw
    xr = x_shard.reshape(b, 2, P, HW).transpose(0, 2, 1, 3).reshape(b, P, 2 * HW)
    x_dev = np.ascontiguousarray(xr).astype(bf16)
    # w_se: [(g,h), n] with flat = c*256 + o = g*8192 + n, n = cl*256 + o
    w_r = w_se_out.reshape(O, C, HID).transpose(1, 0, 2)  # [c, o, h]
    w_r = w_r.reshape(G, ADN, HID).transpose(0, 2, 1).reshape(P, ADN)
    wse_dev = np.ascontiguousarray(w_r).astype(bf16)
    # w_conv in per-sample lhsT layout: [c_p, k*256 + o] = w_conv[o, k*128 + c_p]
    wc = w_conv[:, :, 0, 0]  # [O, C]
    wconv_dev = np.ascontiguousarray(
        wc.T.reshape(2, P, O).transpose(1, 0, 2).reshape(P, 2 * O)
    ).astype(bf16)
    # w_se1 replicated for the G h-groups:
    # [p, k*128 + (g*16+h)] = w_se1[h, k*128+p] / 3136
    w1 = (w_se1.T / float(HW)).reshape(2, P, HID)  # [k, p, h]
    w1 = np.broadcast_to(w1[:, :, None, :], (2, P, G, HID)).reshape(2, P, P)
    w1 = np.ascontiguousarray(w1.transpose(1, 0, 2).reshape(P, 2 * P)).astype(
        np.float32
    )
    # bd mask: [(g,h), b*8 + g'] = 1 if g == g'
    m = np.zeros((G, HID, B_PER_CORE, G), np.float32)
    for g in range(G):
        m[g, :, :, g] = 1.0
    mask_dev = m.reshape(P, B_PER_CORE * G).astype(bf16)
    return {
        "x": x_dev,
        "w_se": wse_dev,
        "w_conv": wconv_dev,
        "w_se1": w1,
        "bd_mask": mask_dev,
    }


def postprocess(raw_out):
    """raw_out: [4, 2, 128, 3136] bf16 -> [4, 256, 56, 56] f32."""
    return np.asarray(raw_out, dtype=np.float32).reshape(B_PER_CORE, O, 56, 56)


_NC_CACHE = None
LAST_RESULT = None


def kernel(x, w_conv, w_se1, w_se_out):
    global _NC_CACHE
    from concourse.bass_utils import run_bass_kernel_spmd

    if _NC_CACHE is None:
        _NC_CACHE = build_nc()
    nc = _NC_CACHE

    B = x.shape[0]
    in_maps = []
    for i in range(N_CORES):
        shard = x[i * B_PER_CORE : (i + 1) * B_PER_CORE]
        in_maps.append(prep_core_inputs(shard, w_conv, w_se1, w_se_out))

    global LAST_RESULT
    res = run_bass_kernel_spmd(nc, in_maps, core_ids=list(range(N_CORES)))
    LAST_RESULT = res
    out = np.concatenate(
        [postprocess(res.results[i]["out"]) for i in range(N_CORES)], axis=0
    )
    assert out.shape == (B, O, 56, 56)
    return out
